# revision 8
# baseline (speedup 1.0000x reference)
"""Trainium2 Bass kernel for nn_CrossHyperedgeGen (moe_routing).

Strategy (8 NeuronCores):
  - Data-parallel over batch B=8: core j owns batch j end-to-end.
  - The large context_net weight Wc [E*D, 4D] = 134MB is tensor-parallel
    sharded over E: core j holds Wc rows for hyperedges [64j, 64j+64).
  - Per-core flow:
      1. ctx reductions (sum/max over nodes) on X^T tiles  -> ctx [1024]
      2. tiny AllGather of ctx across cores
      3. TP matmul: offsets shard for ALL batches (ctx stationary, Wc^T
         streamed; bias bc + prototype_base folded in as a K=1 matmul row)
      4. AllToAll redistributes: core j receives all E-shards of ITS batch
      5. PE-transpose shards into P^T [d, e]
      6. Xp^T = (WA/s)^T-matmul over X^T (bias K=1 row); logits tiles
         [128 nodes, 512 e] = Xp^T-stationary matmul vs P^T
      7. top-128-of-512 per row via DVE max/max_index/match_replace
         (8 at a time, 16 rounds), softmax on the scalar engine
  - Host pre-transposes X/Wc so every DMA is contiguous and every matmul
    contraction dim sits on SBUF partitions; host folds 1/(sqrt(dh)*H)
    into WA/bA/WB/bB and 1/N mean scaling into Wc rows.

kernel(**inputs) takes the FULL unsharded inputs and returns
(topiA, wA, topiB, wB) exactly like the reference.
"""

import numpy as np

import concourse.bacc as bacc
import concourse.mybir as mybir
from concourse import tile
from concourse.bass_utils import run_bass_kernel_spmd

F32 = mybir.dt.float32
U32 = mybir.dt.uint32
AX = mybir.AxisListType.X

N_CORES = 8
B = 8
NA = 4096
NB = 2048
D = 256
E = 512
H = 8
DH = D // H
KE = 128
ESH = E // N_CORES          # 64 hyperedges per shard
EDSH = ESH * D              # 16384
K4D = 4 * D                 # 1024
SCALING = float(np.sqrt(DH)) * H   # divide logits by this (folded into WA/bA)
NEG_BIG = -3.0e38

_CACHE = {}


def _build():
    if "nc" in _CACHE:
        return _CACHE["nc"]

    nc = bacc.Bacc("TRN2", target_bir_lowering=False, debug=False,
                   num_devices=N_CORES)

    # ---- DRAM I/O (per core) ----
    xa_t = nc.dram_tensor("xa_t", [D, NA], F32, kind="ExternalInput")
    xb_t = nc.dram_tensor("xb_t", [D, NB], F32, kind="ExternalInput")
    wc_t = nc.dram_tensor("wc_t", [K4D, EDSH], F32, kind="ExternalInput")
    bias_sh = nc.dram_tensor("bias_sh", [1, EDSH], F32, kind="ExternalInput")
    wa_t = nc.dram_tensor("wa_t", [D, D], F32, kind="ExternalInput")
    ba_s = nc.dram_tensor("ba_s", [1, D], F32, kind="ExternalInput")
    wb_t = nc.dram_tensor("wb_t", [D, D], F32, kind="ExternalInput")
    bb_s = nc.dram_tensor("bb_s", [1, D], F32, kind="ExternalInput")
    ident_in = nc.dram_tensor("ident_in", [128, 128], F32, kind="ExternalInput")

    topia = nc.dram_tensor("topia", [NA, KE], U32, kind="ExternalOutput")
    w_a = nc.dram_tensor("w_a", [NA, KE], F32, kind="ExternalOutput")
    topib = nc.dram_tensor("topib", [NB, KE], U32, kind="ExternalOutput")
    w_b = nc.dram_tensor("w_b", [NB, KE], F32, kind="ExternalOutput")

    with tile.TileContext(nc) as tc:
        with (
            tc.tile_pool(name="xt", bufs=1) as xt_pool,
            tc.tile_pool(name="xpt", bufs=1) as xpt_pool,
            tc.tile_pool(name="consts", bufs=1) as const_pool,
            tc.tile_pool(name="wc_stream", bufs=6) as wc_pool,
            tc.tile_pool(name="small", bufs=2) as small_pool,
            tc.tile_pool(name="topk", bufs=4) as topk_pool,
            tc.tile_pool(name="outs", bufs=4) as out_pool,
            tc.tile_pool(name="psum_off", bufs=2, space="PSUM") as psum_off,
            tc.tile_pool(name="psum_tr", bufs=2, space="PSUM") as psum_tr,
            tc.tile_pool(name="psum_mm", bufs=2, space="PSUM") as psum_mm,
            tc.tile_pool(name="psum_l", bufs=2, space="PSUM") as psum_l_pool,
            tc.tile_pool(name="dram", bufs=1, space="DRAM") as dram_pool,
        ):
            # ---- load X^T (d on partitions), weights, constants ----
            xa = [xt_pool.tile([128, NA], F32, tag=f"xa{t}", name=f"xa{t}") for t in range(2)]
            xb = [xt_pool.tile([128, NB], F32, tag=f"xb{t}", name=f"xb{t}") for t in range(2)]
            for t in range(2):
                nc.sync.dma_start(xa[t][:], xa_t.ap()[t * 128:(t + 1) * 128, :])
                nc.sync.dma_start(xb[t][:], xb_t.ap()[t * 128:(t + 1) * 128, :])

            wa_sb = [const_pool.tile([128, D], F32, tag=f"wa{t}", name=f"wa{t}")
                     for t in range(2)]
            wb_sb = [const_pool.tile([128, D], F32, tag=f"wb{t}", name=f"wb{t}")
                     for t in range(2)]
            ba_sb = const_pool.tile([1, D], F32, tag="ba")
            bb_sb = const_pool.tile([1, D], F32, tag="bb")
            ident = const_pool.tile([128, 128], F32, tag="ident")
            ones8 = const_pool.tile([1, 8], F32, tag="ones8")
            ones512 = const_pool.tile([1, 512], F32, tag="ones512")
            for t in range(2):
                nc.sync.dma_start(wa_sb[t][:],
                                  wa_t.ap()[t * 128:(t + 1) * 128, :])
                nc.sync.dma_start(wb_sb[t][:],
                                  wb_t.ap()[t * 128:(t + 1) * 128, :])
            nc.sync.dma_start(ba_sb[:], ba_s.ap())
            nc.sync.dma_start(bb_sb[:], bb_s.ap())
            nc.sync.dma_start(ident[:], ident_in.ap())
            nc.vector.memset(ones8[:], 1.0)
            nc.vector.memset(ones512[:], 1.0)

            # ---- ctx: per-d sum & max over nodes ----
            # layout [128, 8]: col j = ctx dims [128j, 128j+128)
            # order: sumA(2) maxA(2) sumB(2) maxB(2)
            ctx_own = const_pool.tile([128, 8], F32, tag="ctx")
            scratch = const_pool.tile([128, NA], F32, tag="scratch")
            ident_fn = mybir.ActivationFunctionType.Identity
            for t in range(2):
                nc.scalar.activation(scratch[:], xa[t][:], ident_fn,
                                     accum_out=ctx_own[:, t:t + 1])
                nc.vector.reduce_max(ctx_own[:, 2 + t:3 + t], xa[t][:], axis=AX)
                nc.scalar.activation(scratch[:, 0:NB], xb[t][:], ident_fn,
                                     accum_out=ctx_own[:, 4 + t:5 + t])
                nc.vector.reduce_max(ctx_own[:, 6 + t:7 + t], xb[t][:], axis=AX)

            # ---- collective 1: AllGather ctx ----
            ctx_bounce = dram_pool.tile([8, 128], F32, tag="ctxb")
            ctx_ag = dram_pool.tile([64, 128], F32, tag="ctxag")
            nc.sync.dma_start(ctx_bounce[:].rearrange("a b -> b a"), ctx_own[:])
            nc.gpsimd.collective_compute(
                "AllGather", mybir.AluOpType.bypass,
                replica_groups=[list(range(N_CORES))],
                ins=[ctx_bounce[:].opt()], outs=[ctx_ag[:].opt()],
            )
            # lhsT k-tile j: ctx_all[c=128j+p, b] = ctx_ag[b*8 + j, p]
            ctx_all = const_pool.tile([128, 8, 8], F32, tag="ctxall")  # [p, b, j]
            nc.sync.dma_start(
                ctx_all[:],
                ctx_ag[:].rearrange("(b j) p -> p b j", j=8),
            )

            # ---- TP offsets matmul in two e_local halves, pipelined with
            # AllToAll + P^T assembly so collectives hide under the matmul ----
            HEDS = EDSH // 2   # 8192 = 32 e_local x 256 d per half
            pt = [const_pool.tile([128, E], F32, tag=f"pt{t}", name=f"pt{t}") for t in range(2)]
            off_bounce = [dram_pool.tile([8, HEDS], F32, tag=f"offb{h}",
                                         name=f"offb{h}") for h in range(2)]
            proto_bounce = [dram_pool.tile([8, HEDS], F32, tag=f"protob{h}",
                                           name=f"protob{h}") for h in range(2)]
            for h in range(2):
                for hc in range(HEDS // 512):
                    cn = h * (HEDS // 512) + hc
                    ps = psum_off.tile([8, 512], F32, tag="psoff", name="psoff")
                    for j in range(8):
                        wtile = wc_pool.tile([128, 512], F32, tag="wc",
                                             name="wc")
                        nc.sync.dma_start(
                            wtile[:],
                            wc_t.ap()[j * 128:(j + 1) * 128,
                                      cn * 512:(cn + 1) * 512],
                        )
                        nc.tensor.matmul(ps[:], lhsT=ctx_all[:, :, j],
                                         rhs=wtile[:],
                                         start=(j == 0), stop=False)
                    btile = small_pool.tile([1, 512], F32, tag="btile",
                                            name="btile")
                    nc.sync.dma_start(btile[:],
                                      bias_sh.ap()[:, cn * 512:(cn + 1) * 512])
                    nc.tensor.matmul(ps[:], lhsT=ones8[:], rhs=btile[:],
                                     start=False, stop=True)
                    ev = small_pool.tile([8, 512], F32, tag="offev",
                                         name="ev")
                    nc.scalar.copy(ev[:], ps[:])
                    nc.sync.dma_start(
                        off_bounce[h][:, hc * 512:(hc + 1) * 512], ev[:])

                nc.gpsimd.collective_compute(
                    "AllToAll", mybir.AluOpType.bypass,
                    replica_groups=[list(range(N_CORES))],
                    ins=[off_bounce[h][:].opt()],
                    outs=[proto_bounce[h][:].opt()],
                )
                # P^T assembly for this half: e_local in [32h, 32h+32)
                EH = ESH // 2  # 32
                for s in range(8):
                    p_nat = small_pool.tile([EH, D], F32, tag="pnat",
                                            name="p_nat")
                    nc.sync.dma_start(
                        p_nat[:],
                        proto_bounce[h][:].rearrange(
                            "s (e d) -> s e d", d=D)[s],
                    )
                    for t in range(2):
                        pst = psum_tr.tile([128, EH], F32, tag="pstr",
                                           name="pst")
                        nc.tensor.transpose(
                            pst[:], p_nat[:, t * 128:(t + 1) * 128],
                            ident[0:EH, 0:EH],
                        )
                        e0 = s * ESH + h * EH
                        nc.scalar.copy(pt[t][:, e0:e0 + EH], pst[:])

            # ---- Xp^T matmuls ----
            xpa = [xpt_pool.tile([128, NA], F32, tag=f"xpa{t}", name=f"xpa{t}") for t in range(2)]
            xpb = [xpt_pool.tile([128, NB], F32, tag=f"xpb{t}", name=f"xpb{t}") for t in range(2)]
            for (xsrc, xp, w_sb, b_sb, nn_) in (
                (xa, xpa, wa_sb, ba_sb, NA),
                (xb, xpb, wb_sb, bb_sb, NB),
            ):
                for cn in range(nn_ // 512):
                    for m in range(2):
                        ps = psum_mm.tile([128, 512], F32, tag="psmm")
                        for k in range(2):
                            nc.tensor.matmul(
                                ps[:],
                                lhsT=w_sb[k][:, m * 128:(m + 1) * 128],
                                rhs=xsrc[k][:, cn * 512:(cn + 1) * 512],
                                start=(k == 0), stop=False)
                        nc.tensor.matmul(
                            ps[:], lhsT=b_sb[:, m * 128:(m + 1) * 128],
                            rhs=ones512[:], start=False, stop=True)
                        nc.scalar.copy(xp[m][:, cn * 512:(cn + 1) * 512], ps[:])

            # ---- logits + top-k + softmax + store ----
            tiles = ([("a", mt) for mt in range(NA // 128)]
                     + [("b", mt) for mt in range(NB // 128)])
            for which, mt in tiles:
                xp = xpa if which == "a" else xpb
                ti_out = topia if which == "a" else topib
                w_out = w_a if which == "a" else w_b
                sl = slice(mt * 128, (mt + 1) * 128)
                ps = psum_l_pool.tile([128, E], F32, tag="psl", name="psl")
                for k in range(2):
                    nc.tensor.matmul(ps[:], lhsT=xp[k][:, sl], rhs=pt[k][:],
                                     start=(k == 0), stop=(k == 1))
                v = topk_pool.tile([128, E], F32, tag="v", name="v")
                nc.scalar.copy(v[:], ps[:])
                wv = out_pool.tile([128, KE], F32, tag="wv", name="wv")
                wi = out_pool.tile([128, KE], U32, tag="wi", name="wi")
                for r in range(KE // 8):
                    g8 = wv[:, 8 * r:8 * r + 8]
                    nc.vector.max(out=g8, in_=v[:])
                    nc.vector.max_index(out=wi[:, 8 * r:8 * r + 8],
                                        in_max=g8, in_values=v[:])
                    nc.vector.match_replace(out=v[:], in_to_replace=g8,
                                            in_values=v[:], imm_value=NEG_BIG)
                # softmax over wv rows (wv sorted desc; wv[:,0] is max)
                negm = out_pool.tile([128, 1], F32, tag="negm", name="negm")
                nc.scalar.mul(negm[:], wv[:, 0:1], -1.0)
                wexp = out_pool.tile([128, KE], F32, tag="wexp", name="wexp")
                sums = out_pool.tile([128, 1], F32, tag="sums", name="sums")
                nc.scalar.activation(wexp[:], wv[:],
                                     mybir.ActivationFunctionType.Exp,
                                     bias=negm[:], scale=1.0,
                                     accum_out=sums[:])
                rec = out_pool.tile([128, 1], F32, tag="rec", name="rec")
                nc.vector.reciprocal(rec[:], sums[:])
                nc.scalar.mul(wexp[:], wexp[:], rec[:])
                nc.sync.dma_start(ti_out.ap()[sl, :], wi[:])
                nc.sync.dma_start(w_out.ap()[sl, :], wexp[:])

    nc.compile()
    _CACHE["nc"] = nc
    return nc


def _prep_inputs(X_A, X_B, prototype_base, Wc, bc, WA, bA, WB, bB):
    """Host-side sharding + layout prep. Returns in_maps for 8 cores."""
    X_A = np.asarray(X_A, dtype=np.float32)
    X_B = np.asarray(X_B, dtype=np.float32)
    prototype_base = np.asarray(prototype_base, dtype=np.float32)
    Wc = np.asarray(Wc, dtype=np.float32)
    bc = np.asarray(bc, dtype=np.float32)
    WA = np.asarray(WA, dtype=np.float32)
    bA = np.asarray(bA, dtype=np.float32)
    WB = np.asarray(WB, dtype=np.float32)
    bB = np.asarray(bB, dtype=np.float32)

    ident = np.eye(128, dtype=np.float32)
    wa_t = np.ascontiguousarray(WA.T) / np.float32(SCALING)
    wb_t = np.ascontiguousarray(WB.T) / np.float32(SCALING)
    ba_s = (bA / np.float32(SCALING)).reshape(1, D)
    bb_s = (bB / np.float32(SCALING)).reshape(1, D)

    in_maps = []
    for j in range(N_CORES):
        sl = slice(j * EDSH, (j + 1) * EDSH)
        wc_sh_t = np.ascontiguousarray(Wc[sl, :].T)  # [1024, 16384]
        # fold the 1/N of the mean context features into Wc rows
        wc_sh_t[0:D, :] *= np.float32(1.0 / NA)
        wc_sh_t[2 * D:3 * D, :] *= np.float32(1.0 / NB)
        bias_sh = (bc[sl] + prototype_base[j * ESH:(j + 1) * ESH, :].ravel()
                   ).reshape(1, EDSH)
        in_maps.append({
            "xa_t": np.ascontiguousarray(X_A[j].T),
            "xb_t": np.ascontiguousarray(X_B[j].T),
            "wc_t": wc_sh_t,
            "bias_sh": np.ascontiguousarray(bias_sh),
            "wa_t": wa_t, "ba_s": ba_s, "wb_t": wb_t, "bb_s": bb_s,
            "ident_in": ident,
        })
    return in_maps


def _run(inputs, trace=False):
    nc = _build()
    in_maps = _prep_inputs(**inputs)
    res = run_bass_kernel_spmd(nc, in_maps, core_ids=list(range(N_CORES)),
                               trace=trace)
    topiA = np.stack([res.results[j]["topia"] for j in range(N_CORES)]
                     ).astype(np.int32)
    wA = np.stack([res.results[j]["w_a"] for j in range(N_CORES)])
    topiB = np.stack([res.results[j]["topib"] for j in range(N_CORES)]
                     ).astype(np.int32)
    wB = np.stack([res.results[j]["w_b"] for j in range(N_CORES)])
    return (topiA, wA, topiB, wB), res


def kernel(**inputs):
    out, _ = _run(inputs, trace=False)
    return out


# revision 9
# speedup vs baseline: 1.0442x; 1.0442x over previous
"""Trainium2 Bass kernel for nn_CrossHyperedgeGen (moe_routing).

Strategy (8 NeuronCores):
  - Data-parallel over batch B=8: core j owns batch j end-to-end.
  - The large context_net weight Wc [E*D, 4D] = 134MB is tensor-parallel
    sharded over E: core j holds Wc rows for hyperedges [64j, 64j+64).
  - Per-core flow:
      1. ctx reductions (sum/max over nodes) on X^T tiles  -> ctx [1024]
      2. tiny AllGather of ctx across cores
      3. TP matmul: offsets shard for ALL batches (ctx stationary, Wc^T
         streamed; bias bc + prototype_base folded in as a K=1 matmul row)
      4. AllToAll redistributes: core j receives all E-shards of ITS batch
      5. PE-transpose shards into P^T [d, e]
      6. Xp^T = (WA/s)^T-matmul over X^T (bias K=1 row); logits tiles
         [128 nodes, 512 e] = Xp^T-stationary matmul vs P^T
      7. top-128-of-512 per row via DVE max/max_index/match_replace
         (8 at a time, 16 rounds), softmax on the scalar engine
  - Host pre-transposes X/Wc so every DMA is contiguous and every matmul
    contraction dim sits on SBUF partitions; host folds 1/(sqrt(dh)*H)
    into WA/bA/WB/bB and 1/N mean scaling into Wc rows.

kernel(**inputs) takes the FULL unsharded inputs and returns
(topiA, wA, topiB, wB) exactly like the reference.
"""

import numpy as np

import concourse.bacc as bacc
import concourse.mybir as mybir
from concourse import tile
from concourse.bass_utils import run_bass_kernel_spmd

F32 = mybir.dt.float32
U32 = mybir.dt.uint32
AX = mybir.AxisListType.X

N_CORES = 8
B = 8
NA = 4096
NB = 2048
D = 256
E = 512
H = 8
DH = D // H
KE = 128
ESH = E // N_CORES          # 64 hyperedges per shard
EDSH = ESH * D              # 16384
K4D = 4 * D                 # 1024
SCALING = float(np.sqrt(DH)) * H   # divide logits by this (folded into WA/bA)
NEG_BIG = -3.0e38

_CACHE = {}


def _build():
    if "nc" in _CACHE:
        return _CACHE["nc"]

    nc = bacc.Bacc("TRN2", target_bir_lowering=False, debug=False,
                   num_devices=N_CORES)

    # ---- DRAM I/O (per core) ----
    xa_t = nc.dram_tensor("xa_t", [D, NA], F32, kind="ExternalInput")
    xb_t = nc.dram_tensor("xb_t", [D, NB], F32, kind="ExternalInput")
    wc_t = nc.dram_tensor("wc_t", [K4D, EDSH], F32, kind="ExternalInput")
    bias_sh = nc.dram_tensor("bias_sh", [1, EDSH], F32, kind="ExternalInput")
    wa_t = nc.dram_tensor("wa_t", [D, D], F32, kind="ExternalInput")
    ba_s = nc.dram_tensor("ba_s", [1, D], F32, kind="ExternalInput")
    wb_t = nc.dram_tensor("wb_t", [D, D], F32, kind="ExternalInput")
    bb_s = nc.dram_tensor("bb_s", [1, D], F32, kind="ExternalInput")
    ident_in = nc.dram_tensor("ident_in", [128, 128], F32, kind="ExternalInput")

    topia = nc.dram_tensor("topia", [NA, KE], U32, kind="ExternalOutput")
    w_a = nc.dram_tensor("w_a", [NA, KE], F32, kind="ExternalOutput")
    topib = nc.dram_tensor("topib", [NB, KE], U32, kind="ExternalOutput")
    w_b = nc.dram_tensor("w_b", [NB, KE], F32, kind="ExternalOutput")

    with tile.TileContext(nc) as tc:
        with (
            tc.tile_pool(name="xt", bufs=1) as xt_pool,
            tc.tile_pool(name="xpt", bufs=1) as xpt_pool,
            tc.tile_pool(name="consts", bufs=1) as const_pool,
            tc.tile_pool(name="wc_stream", bufs=6) as wc_pool,
            tc.tile_pool(name="small", bufs=2) as small_pool,
            tc.tile_pool(name="topk", bufs=4) as topk_pool,
            tc.tile_pool(name="outs", bufs=4) as out_pool,
            tc.tile_pool(name="psum_off", bufs=2, space="PSUM") as psum_off,
            tc.tile_pool(name="psum_tr", bufs=2, space="PSUM") as psum_tr,
            tc.tile_pool(name="psum_mm", bufs=2, space="PSUM") as psum_mm,
            tc.tile_pool(name="psum_l", bufs=2, space="PSUM") as psum_l_pool,
            tc.tile_pool(name="dram", bufs=1, space="DRAM") as dram_pool,
        ):
            # ---- load X^T (d on partitions), weights, constants ----
            XCH = 1024  # DMA/reduce chunk so ctx reductions ride the stream
            xa = [xt_pool.tile([128, NA], F32, tag=f"xa{t}", name=f"xa{t}") for t in range(2)]
            xb = [xt_pool.tile([128, NB], F32, tag=f"xb{t}", name=f"xb{t}") for t in range(2)]
            for t in range(2):
                for c0 in range(0, NA, XCH):
                    nc.sync.dma_start(xa[t][:, c0:c0 + XCH],
                                      xa_t.ap()[t * 128:(t + 1) * 128,
                                                c0:c0 + XCH])
                for c0 in range(0, NB, XCH):
                    nc.sync.dma_start(xb[t][:, c0:c0 + XCH],
                                      xb_t.ap()[t * 128:(t + 1) * 128,
                                                c0:c0 + XCH])

            wa_sb = [const_pool.tile([128, D], F32, tag=f"wa{t}", name=f"wa{t}")
                     for t in range(2)]
            wb_sb = [const_pool.tile([128, D], F32, tag=f"wb{t}", name=f"wb{t}")
                     for t in range(2)]
            ba_sb = const_pool.tile([1, D], F32, tag="ba")
            bb_sb = const_pool.tile([1, D], F32, tag="bb")
            ident = const_pool.tile([128, 128], F32, tag="ident")
            ones8 = const_pool.tile([1, 8], F32, tag="ones8")
            ones512 = const_pool.tile([1, 512], F32, tag="ones512")
            for t in range(2):
                nc.sync.dma_start(wa_sb[t][:],
                                  wa_t.ap()[t * 128:(t + 1) * 128, :])
                nc.sync.dma_start(wb_sb[t][:],
                                  wb_t.ap()[t * 128:(t + 1) * 128, :])
            nc.sync.dma_start(ba_sb[:], ba_s.ap())
            nc.sync.dma_start(bb_sb[:], bb_s.ap())
            nc.sync.dma_start(ident[:], ident_in.ap())
            nc.vector.memset(ones8[:], 1.0)
            nc.vector.memset(ones512[:], 1.0)

            # ---- ctx: per-d sum & max over nodes ----
            # layout [128, 8]: col j = ctx dims [128j, 128j+128)
            # order: sumA(2) maxA(2) sumB(2) maxB(2)
            ctx_own = const_pool.tile([128, 8], F32, tag="ctx")
            scratch = const_pool.tile([128, XCH], F32, tag="scratch")
            partial = const_pool.tile([128, 12], F32, tag="partial")
            ident_fn = mybir.ActivationFunctionType.Identity
            for t in range(2):
                # per-chunk partial sums (scalar engine) and maxes (vector)
                pa_s = partial[:, 0:4]
                pa_m = partial[:, 4:8]
                for ci, c0 in enumerate(range(0, NA, XCH)):
                    nc.scalar.activation(scratch[:], xa[t][:, c0:c0 + XCH],
                                         ident_fn,
                                         accum_out=pa_s[:, ci:ci + 1])
                    nc.vector.reduce_max(pa_m[:, ci:ci + 1],
                                         xa[t][:, c0:c0 + XCH], axis=AX)
                pb_s = partial[:, 8:10]
                pb_m = partial[:, 10:12]
                for ci, c0 in enumerate(range(0, NB, XCH)):
                    nc.scalar.activation(scratch[:], xb[t][:, c0:c0 + XCH],
                                         ident_fn,
                                         accum_out=pb_s[:, ci:ci + 1])
                    nc.vector.reduce_max(pb_m[:, ci:ci + 1],
                                         xb[t][:, c0:c0 + XCH], axis=AX)
                nc.vector.reduce_sum(ctx_own[:, t:t + 1], pa_s, axis=AX)
                nc.vector.reduce_max(ctx_own[:, 2 + t:3 + t], pa_m, axis=AX)
                nc.vector.reduce_sum(ctx_own[:, 4 + t:5 + t], pb_s, axis=AX)
                nc.vector.reduce_max(ctx_own[:, 6 + t:7 + t], pb_m, axis=AX)

            # ---- collective 1: AllGather ctx ----
            ctx_bounce = dram_pool.tile([8, 128], F32, tag="ctxb")
            ctx_ag = dram_pool.tile([64, 128], F32, tag="ctxag")
            nc.sync.dma_start(ctx_bounce[:].rearrange("a b -> b a"), ctx_own[:])
            nc.gpsimd.collective_compute(
                "AllGather", mybir.AluOpType.bypass,
                replica_groups=[list(range(N_CORES))],
                ins=[ctx_bounce[:].opt()], outs=[ctx_ag[:].opt()],
            )
            # lhsT k-tile j: ctx_all[c=128j+p, b] = ctx_ag[b*8 + j, p]
            ctx_all = const_pool.tile([128, 8, 8], F32, tag="ctxall")  # [p, b, j]
            nc.sync.dma_start(
                ctx_all[:],
                ctx_ag[:].rearrange("(b j) p -> p b j", j=8),
            )

            # ---- TP offsets matmul in two e_local halves, pipelined with
            # AllToAll + P^T assembly so collectives hide under the matmul ----
            HEDS = EDSH // 2   # 8192 = 32 e_local x 256 d per half
            pt = [const_pool.tile([128, E], F32, tag=f"pt{t}", name=f"pt{t}") for t in range(2)]
            off_bounce = [dram_pool.tile([8, HEDS], F32, tag=f"offb{h}",
                                         name=f"offb{h}") for h in range(2)]
            proto_bounce = [dram_pool.tile([8, HEDS], F32, tag=f"protob{h}",
                                           name=f"protob{h}") for h in range(2)]
            for h in range(2):
                for hc in range(HEDS // 512):
                    cn = h * (HEDS // 512) + hc
                    ps = psum_off.tile([8, 512], F32, tag="psoff", name="psoff")
                    for j in range(8):
                        wtile = wc_pool.tile([128, 512], F32, tag="wc",
                                             name="wc")
                        nc.sync.dma_start(
                            wtile[:],
                            wc_t.ap()[j * 128:(j + 1) * 128,
                                      cn * 512:(cn + 1) * 512],
                        )
                        nc.tensor.matmul(ps[:], lhsT=ctx_all[:, :, j],
                                         rhs=wtile[:],
                                         start=(j == 0), stop=False)
                    btile = small_pool.tile([1, 512], F32, tag="btile",
                                            name="btile")
                    nc.sync.dma_start(btile[:],
                                      bias_sh.ap()[:, cn * 512:(cn + 1) * 512])
                    nc.tensor.matmul(ps[:], lhsT=ones8[:], rhs=btile[:],
                                     start=False, stop=True)
                    ev = small_pool.tile([8, 512], F32, tag="offev",
                                         name="ev")
                    nc.scalar.copy(ev[:], ps[:])
                    nc.sync.dma_start(
                        off_bounce[h][:, hc * 512:(hc + 1) * 512], ev[:])

                nc.gpsimd.collective_compute(
                    "AllToAll", mybir.AluOpType.bypass,
                    replica_groups=[list(range(N_CORES))],
                    ins=[off_bounce[h][:].opt()],
                    outs=[proto_bounce[h][:].opt()],
                )
                # P^T assembly for this half: e_local in [32h, 32h+32)
                EH = ESH // 2  # 32
                for s in range(8):
                    p_nat = small_pool.tile([EH, D], F32, tag="pnat",
                                            name="p_nat")
                    nc.sync.dma_start(
                        p_nat[:],
                        proto_bounce[h][:].rearrange(
                            "s (e d) -> s e d", d=D)[s],
                    )
                    for t in range(2):
                        pst = psum_tr.tile([128, EH], F32, tag="pstr",
                                           name="pst")
                        nc.tensor.transpose(
                            pst[:], p_nat[:, t * 128:(t + 1) * 128],
                            ident[0:EH, 0:EH],
                        )
                        e0 = s * ESH + h * EH
                        nc.scalar.copy(pt[t][:, e0:e0 + EH], pst[:])

            # ---- Xp^T matmuls ----
            xpa = [xpt_pool.tile([128, NA], F32, tag=f"xpa{t}", name=f"xpa{t}") for t in range(2)]
            xpb = [xpt_pool.tile([128, NB], F32, tag=f"xpb{t}", name=f"xpb{t}") for t in range(2)]
            for (xsrc, xp, w_sb, b_sb, nn_) in (
                (xa, xpa, wa_sb, ba_sb, NA),
                (xb, xpb, wb_sb, bb_sb, NB),
            ):
                for cn in range(nn_ // 512):
                    for m in range(2):
                        ps = psum_mm.tile([128, 512], F32, tag="psmm")
                        for k in range(2):
                            nc.tensor.matmul(
                                ps[:],
                                lhsT=w_sb[k][:, m * 128:(m + 1) * 128],
                                rhs=xsrc[k][:, cn * 512:(cn + 1) * 512],
                                start=(k == 0), stop=False)
                        nc.tensor.matmul(
                            ps[:], lhsT=b_sb[:, m * 128:(m + 1) * 128],
                            rhs=ones512[:], start=False, stop=True)
                        nc.scalar.copy(xp[m][:, cn * 512:(cn + 1) * 512], ps[:])

            # ---- logits + top-k + softmax + store ----
            tiles = ([("a", mt) for mt in range(NA // 128)]
                     + [("b", mt) for mt in range(NB // 128)])
            for which, mt in tiles:
                xp = xpa if which == "a" else xpb
                ti_out = topia if which == "a" else topib
                w_out = w_a if which == "a" else w_b
                sl = slice(mt * 128, (mt + 1) * 128)
                ps = psum_l_pool.tile([128, E], F32, tag="psl", name="psl")
                for k in range(2):
                    nc.tensor.matmul(ps[:], lhsT=xp[k][:, sl], rhs=pt[k][:],
                                     start=(k == 0), stop=(k == 1))
                v = topk_pool.tile([128, E], F32, tag="v", name="v")
                nc.scalar.copy(v[:], ps[:])
                wv = out_pool.tile([128, KE], F32, tag="wv", name="wv")
                wi = out_pool.tile([128, KE], U32, tag="wi", name="wi")
                for r in range(KE // 8):
                    g8 = wv[:, 8 * r:8 * r + 8]
                    nc.vector.max(out=g8, in_=v[:])
                    nc.vector.max_index(out=wi[:, 8 * r:8 * r + 8],
                                        in_max=g8, in_values=v[:])
                    if r < KE // 8 - 1:  # nothing reads v after the last round
                        nc.vector.match_replace(out=v[:], in_to_replace=g8,
                                                in_values=v[:],
                                                imm_value=NEG_BIG)
                # softmax over wv rows (wv sorted desc; wv[:,0] is max)
                negm = out_pool.tile([128, 1], F32, tag="negm", name="negm")
                nc.scalar.mul(negm[:], wv[:, 0:1], -1.0)
                wexp = out_pool.tile([128, KE], F32, tag="wexp", name="wexp")
                sums = out_pool.tile([128, 1], F32, tag="sums", name="sums")
                nc.scalar.activation(wexp[:], wv[:],
                                     mybir.ActivationFunctionType.Exp,
                                     bias=negm[:], scale=1.0,
                                     accum_out=sums[:])
                rec = out_pool.tile([128, 1], F32, tag="rec", name="rec")
                nc.vector.reciprocal(rec[:], sums[:])
                nc.scalar.mul(wexp[:], wexp[:], rec[:])
                nc.sync.dma_start(ti_out.ap()[sl, :], wi[:])
                nc.sync.dma_start(w_out.ap()[sl, :], wexp[:])

    nc.compile()
    _CACHE["nc"] = nc
    return nc


def _prep_inputs(X_A, X_B, prototype_base, Wc, bc, WA, bA, WB, bB):
    """Host-side sharding + layout prep. Returns in_maps for 8 cores."""
    X_A = np.asarray(X_A, dtype=np.float32)
    X_B = np.asarray(X_B, dtype=np.float32)
    prototype_base = np.asarray(prototype_base, dtype=np.float32)
    Wc = np.asarray(Wc, dtype=np.float32)
    bc = np.asarray(bc, dtype=np.float32)
    WA = np.asarray(WA, dtype=np.float32)
    bA = np.asarray(bA, dtype=np.float32)
    WB = np.asarray(WB, dtype=np.float32)
    bB = np.asarray(bB, dtype=np.float32)

    ident = np.eye(128, dtype=np.float32)
    wa_t = np.ascontiguousarray(WA.T) / np.float32(SCALING)
    wb_t = np.ascontiguousarray(WB.T) / np.float32(SCALING)
    ba_s = (bA / np.float32(SCALING)).reshape(1, D)
    bb_s = (bB / np.float32(SCALING)).reshape(1, D)

    in_maps = []
    for j in range(N_CORES):
        sl = slice(j * EDSH, (j + 1) * EDSH)
        wc_sh_t = np.ascontiguousarray(Wc[sl, :].T)  # [1024, 16384]
        # fold the 1/N of the mean context features into Wc rows
        wc_sh_t[0:D, :] *= np.float32(1.0 / NA)
        wc_sh_t[2 * D:3 * D, :] *= np.float32(1.0 / NB)
        bias_sh = (bc[sl] + prototype_base[j * ESH:(j + 1) * ESH, :].ravel()
                   ).reshape(1, EDSH)
        in_maps.append({
            "xa_t": np.ascontiguousarray(X_A[j].T),
            "xb_t": np.ascontiguousarray(X_B[j].T),
            "wc_t": wc_sh_t,
            "bias_sh": np.ascontiguousarray(bias_sh),
            "wa_t": wa_t, "ba_s": ba_s, "wb_t": wb_t, "bb_s": bb_s,
            "ident_in": ident,
        })
    return in_maps


def _run(inputs, trace=False):
    nc = _build()
    in_maps = _prep_inputs(**inputs)
    res = run_bass_kernel_spmd(nc, in_maps, core_ids=list(range(N_CORES)),
                               trace=trace)
    topiA = np.stack([res.results[j]["topia"] for j in range(N_CORES)]
                     ).astype(np.int32)
    wA = np.stack([res.results[j]["w_a"] for j in range(N_CORES)])
    topiB = np.stack([res.results[j]["topib"] for j in range(N_CORES)]
                     ).astype(np.int32)
    wB = np.stack([res.results[j]["w_b"] for j in range(N_CORES)])
    return (topiA, wA, topiB, wB), res


def kernel(**inputs):
    out, _ = _run(inputs, trace=False)
    return out
